# revision 19
# baseline (speedup 1.0000x reference)
"""Block-circulant linear (MINI_BLOCK=4) via length-4 rFFT factorization on 8 trn2 cores.

Math: out = x @ W^T where W[4y+n, 4x+j] = eigens[y, x, (n-j) mod 4].
In the length-4 DFT domain the circulant contraction factors into 5 real
matmul chains over the block-index axis gx=1024 (Gauss 3-mult for the complex
bin; FLOP-optimal per Winograd for length-4 cyclic convolution):
  X0 = x0+x1+x2+x3, X1 = (x0-x2) + i(x3-x1), X2 = x0-x1+x2-x3  (per block of 4)
  Y0 = E0^T X0, Y2 = E2^T X2, g1 = E1r^T (X1r+X1i), g2 = Ed^T X1r, g3 = Es^T X1i
  Y1r = g1-g3, Y1i = g1+g2
  o0 = Y0+Y1r+Y2, o1 = Y0-Y1i-Y2, o2 = Y0-Y1r+Y2, o3 = Y0+Y1i-Y2  (scales in E)

Sharding: data-parallel over batch, 512 rows per core; E replicated.

Orientation: E is the matmul *stationary* operand ([128x, 128y] chunks), the
DFT'd x is the *moving* operand ([128x, 512b]); each output group is 128
y-blocks x full batch shard, so a group needs only x + 1/8 of E to retire.
Schedule: x split over three HWDGE rings (sync/scalar/vector) so it outruns
the E stream (gpsimd ring, fine-grained first chunk); 16 dependency-free
warmup matmuls ramp the PE p-state; groups 0+1 interleave 8 chains per
xc-step (exactly 8 PSUM banks) to keep the PE fed during the stream chase;
all PSUM drains are fp16 ACT copies (g2/g3 first so the single-buffered
banks recycle just-in-time); DVE runs fp16 2x-mode butterflies/combines;
the last group runs as two half-batch chain sets so the final epilogue is
half-length and overlaps the other half's matmuls. Output is stored
[y, j, b]-packed fp16 and the host transposes/casts back.
"""
import numpy as np

B, IN, OUT, BLK = 4096, 4096, 4096, 4
GX, GY = IN // BLK, OUT // BLK        # 1024, 1024
NCORES = 8
BS = B // NCORES                      # 512 batch rows per core
XC = GX // 128                        # 8 x-chunks (contraction)
YCH = GY // 128                       # 8 y-groups (128 y-blocks each)
YCHP = YCH // 2                       # 4 E-pack chunks (256 y each)

_cache = {}


def _build_nc():
    from concourse import bacc
    import concourse.mybir as mybir
    from concourse.tile import TileContext

    f32 = mybir.dt.float32
    f16 = mybir.dt.float16

    nc = bacc.Bacc("TRN2", target_bir_lowering=False, debug=False,
                   enable_asserts=False, num_devices=NCORES)
    # x shard host-packed: [XC, 128, 4*BS] = (xc, p, (j b)); row 4*(128*xc+p)+j
    # of x^T lands at [xc, p, j*BS:]. 4KB contiguous per partition per chunk.
    xp_d = nc.dram_tensor("xp", [XC, 128, 4 * BS], f16, kind="ExternalInput")
    # E host-packed in consumption order: [YCHP, 128, XC, 5*256] =
    # (ychp, p=x%128, xc, (k y256)); 2.5KB contiguous runs per partition.
    ep_d = nc.dram_tensor("ep", [YCHP, 128, XC, 5 * 256], f16, kind="ExternalInput")
    # out packed [ych, p=y%128, (j b)] fp16; host transposes to [b, 4y+j].
    ob_d = nc.dram_tensor("ob", [YCH, 128, 4 * BS], f16, kind="ExternalOutput")

    with TileContext(nc) as tc:
        with (
            tc.tile_pool(name="xin", bufs=1) as xip,
            tc.tile_pool(name="xt", bufs=1) as xtp,
            tc.tile_pool(name="esl", bufs=1) as esp,
            tc.tile_pool(name="bfly", bufs=2) as bfp,
            tc.tile_pool(name="drain", bufs=2) as drp,
            tc.tile_pool(name="outp", bufs=2) as op_,
            tc.tile_pool(name="wrm", bufs=1) as wmp,
            tc.tile_pool(name="mpsum", bufs=1, space="PSUM") as mps,
        ):
            xin = xip.tile([128, XC, 4 * BS], f16, tag="xin")
            xt = [xtp.tile([128, XC, BS], f16, tag=f"xt{k}", name=f"xt{k}")
                  for k in range(5)]  # X0, X1s, X1r, X2, X1i
            eslab = esp.tile([128, YCHP, XC, 5 * 256], f16, tag="eslab")

            # --- PE p-state warmup FIRST: dependency-free matmuls on a zeroed
            # tile, alternating the two single-buffered PSUM tags (banks are
            # WAW-recycled by group 0 with no drain needed). The PE ramps
            # 0.65->1.2->2.4GHz over ~3us of busy, so this hides the ramp
            # under the preamble/stream wait.
            wt = wmp.tile([128, 128 + BS], f16, tag="warm")
            nc.gpsimd.memset(wt, 0.0)
            for i in range(14):
                wp = mps.tile([128, BS], f32, tag=("g2" if i % 2 else "g3"),
                              name="wp")
                nc.tensor.matmul(wp, wt[:, :128], wt[:, 128:], start=True, stop=True)

            # --- DMA issues: x and the first E chunk interleaved per-xc in
            # exact phase-1 consumption order, split over the two HWDGE rings
            # (sync even xc, scalar odd xc); the later E chunks follow.
            for xc in range(XC):
                ring = nc.sync if xc % 2 == 0 else nc.scalar
                ring.dma_start(out=xin[:, xc], in_=xp_d[xc])
                ring.dma_start(out=eslab[:, 0, xc], in_=ep_d[0, :, xc])
            for ring, ychp, xlo, xhi in ((nc.sync, 1, 0, 2), (nc.scalar, 1, 2, 4),
                                         (nc.sync, 1, 4, 6), (nc.scalar, 1, 6, 8),
                                         (nc.sync, 2, 0, 8), (nc.scalar, 3, 0, 8)):
                ring.dma_start(out=eslab[:, ychp, xlo:xhi],
                               in_=ep_d[ychp, :, xlo:xhi])

            # --- Forward DFT butterflies, fp16 DVE, chasing the x stream.
            for xc in range(XC):
                xj = [xin[:, xc, j * BS:(j + 1) * BS] for j in range(4)]
                s02 = bfp.tile([128, BS], f16, tag="s02")
                s13 = bfp.tile([128, BS], f16, tag="s13")
                nc.vector.tensor_add(out=s02, in0=xj[0], in1=xj[2])
                nc.vector.tensor_add(out=s13, in0=xj[1], in1=xj[3])
                nc.vector.tensor_sub(out=xt[2][:, xc], in0=xj[0], in1=xj[2])
                nc.vector.tensor_sub(out=xt[4][:, xc], in0=xj[3], in1=xj[1])
                nc.vector.tensor_add(out=xt[0][:, xc], in0=s02, in1=s13)
                nc.vector.tensor_sub(out=xt[3][:, xc], in0=s02, in1=s13)
                nc.vector.tensor_add(out=xt[1][:, xc], in0=xt[2][:, xc], in1=xt[4][:, xc])

            # --- Main loop helpers. Chain order i: y0, g1, y2, g2, g3;
            # E-matrix pack order k: E0, E1r, Ed, E2, Es; chain i uses E-matrix
            # kmap[i] against xt[kmap'[i]] (same permutation both sides).
            kmap = (0, 1, 3, 2, 4)

            def alloc_g23():
                return (mps.tile([128, BS], f32, tag="g2", name="g2"),
                        mps.tile([128, BS], f32, tag="g3", name="g3"))

            def alloc_chains(dbl_only=False):
                y0 = mps.tile([128, BS], f32, tag="y0", bufs=2)
                g1 = mps.tile([128, BS], f32, tag="g1", bufs=2)
                y2 = mps.tile([128, BS], f32, tag="y2", bufs=2)
                if dbl_only:
                    return (y0, g1, y2)
                return (y0, g1, y2) + alloc_g23()

            def mm(chains, ych, i, xc, bsl, start, stop):
                k = kmap[i]
                lo = k * 256 + (ych % 2) * 128
                nc.tensor.matmul(chains[i][:, bsl],
                                 eslab[:, ych // 2, xc, lo:lo + 128],
                                 xt[k][:, xc, bsl], start=start, stop=stop)

            BSL = slice(0, BS)

            def emit_chains(chains, ych, bsl=BSL, idxs=None):
                idxs = range(len(chains)) if idxs is None else idxs
                # first two xc steps run double-buffered chains first so the
                # PE never waits on the prior group's g2/g3 drains
                for xc in (0, 1):
                    for i in idxs:
                        if i < 3:
                            mm(chains, ych, i, xc, bsl, xc == 0, False)
                for xc in (0, 1):
                    for i in idxs:
                        if i >= 3:
                            mm(chains, ych, i, xc, bsl, xc == 0, False)
                for xc in range(2, XC):
                    for i in idxs:
                        mm(chains, ych, i, xc, bsl, False, xc == XC - 1)

            def emit_epi(chains, ych, bsl=BSL, ot=None):
                y0, g1, y2, g2, g3 = chains
                n = bsl.stop - bsl.start
                cv = {}
                # ACT drains: single-buffered banks first, then DVE deps
                for nm, src in (("cg2", g2), ("cg3", g3), ("cy2", y2),
                                ("cy0", y0), ("cg1", g1)):
                    cv[nm] = drp.tile([128, n], f16, tag=nm, name=nm)
                    nc.scalar.copy(out=cv[nm], in_=src[:, bsl])
                a_ = drp.tile([128, n], f16, tag="a")
                b_ = drp.tile([128, n], f16, tag="b")
                c_ = drp.tile([128, n], f16, tag="c")
                d_ = drp.tile([128, n], f16, tag="d")
                nc.vector.tensor_sub(out=c_, in0=cv["cg1"], in1=cv["cg3"])  # Y1r
                nc.vector.tensor_add(out=d_, in0=cv["cg1"], in1=cv["cg2"])  # Y1i
                nc.vector.tensor_add(out=a_, in0=cv["cy0"], in1=cv["cy2"])  # Y0+Y2
                nc.vector.tensor_sub(out=b_, in0=cv["cy0"], in1=cv["cy2"])  # Y0-Y2
                if ot is None:
                    ot = op_.tile([128, 4 * BS], f16, tag="ot")
                otv = ot.rearrange("p (j b) -> p j b", j=4)
                # j planes packed in order (1, 3, 0, 2) so the tail's two
                # store halves are contiguous; _post undoes the permutation
                nc.vector.tensor_add(out=otv[:, 2, bsl], in0=a_, in1=c_)  # o0
                nc.vector.tensor_sub(out=otv[:, 3, bsl], in0=a_, in1=c_)  # o2
                nc.vector.tensor_sub(out=otv[:, 0, bsl], in0=b_, in1=d_)  # o1
                nc.vector.tensor_add(out=otv[:, 1, bsl], in0=b_, in1=d_)  # o3
                if n == BS:
                    nc.sync.dma_start(out=ob_d[ych], in_=ot)
                else:
                    nc.sync.dma_start(
                        out=ob_d[ych].rearrange("p (j b) -> p j b", j=4)[:, :, bsl],
                        in_=otv[:, :, bsl])
                return ot

            # --- Groups 0+1 interleaved: 8 chains per xc-step (8 PSUM banks)
            # so the PE keeps pace with the x/E stream; then ych1's g2/g3.
            # inner order follows butterfly completion (X1r, X1i land first,
            # X1s last) so the first matmul issues as early as possible
            ch0 = alloc_chains()
            ch1d = alloc_chains(dbl_only=True)
            for xc in range(XC):
                st, sp = xc == 0, xc == XC - 1
                for i in (3, 4, 0, 2, 1):
                    mm(ch0, 0, i, xc, BSL, st, sp)
                for i in (0, 2, 1):
                    mm(ch1d, 1, i, xc, BSL, st, sp)
            emit_epi(ch0, 0)
            ch1 = ch1d + alloc_g23()
            emit_chains(ch1, 1, idxs=(3, 4))
            emit_epi(ch1, 1)

            # --- Groups 2..6 plain; group 7 as two half-batch chain sets so
            # the tail epilogue is half-length and overlaps h1's matmuls.
            for ych in range(2, YCH - 1):
                ch = alloc_chains()
                emit_chains(ch, ych)
                emit_epi(ch, ych)
            b0, b1 = slice(0, BS // 2), slice(BS // 2, BS)
            ch = alloc_chains()
            emit_chains(ch, YCH - 1, bsl=b0)
            ot7 = emit_epi(ch, YCH - 1, bsl=b0)
            # h1 tail: chain-major so y0/g1 stop early and drain under the
            # y2/g2/g3 phase; a/b read y2 straight from PSUM. Shortens the
            # post-last-matmul critical path to ~2us + store.
            y0, g1, y2, g2, g3 = alloc_chains()
            ch = (y0, g1, y2, g2, g3)
            for xc in range(XC):
                mm(ch, YCH - 1, 0, xc, b1, xc == 0, xc == XC - 1)
                mm(ch, YCH - 1, 1, xc, b1, xc == 0, xc == XC - 1)
            n2 = BS // 2
            cg1 = drp.tile([128, n2], f16, tag="cg1", name="cg1")
            cy0 = drp.tile([128, n2], f16, tag="cy0", name="cy0")
            nc.scalar.copy(out=cg1, in_=g1[:, b1])
            nc.scalar.copy(out=cy0, in_=y0[:, b1])
            # y2 last in the cycle so g3/g2 stop (and drain) before the final
            # matmul; DVE handles the b/d leg while gpsimd does the c leg.
            for xc in range(XC):
                for i in (4, 3, 2):
                    mm(ch, YCH - 1, i, xc, b1, xc == 0, xc == XC - 1)
            cg3 = drp.tile([128, n2], f16, tag="cg3", name="cg3")
            cg2 = drp.tile([128, n2], f16, tag="cg2", name="cg2")
            nc.scalar.copy(out=cg3, in_=g3[:, b1])
            nc.scalar.copy(out=cg2, in_=g2[:, b1])
            a_ = drp.tile([128, n2], f16, tag="a", name="a")
            b_ = drp.tile([128, n2], f16, tag="b", name="b")
            c_ = drp.tile([128, n2], f16, tag="c", name="c")
            d_ = drp.tile([128, n2], f16, tag="d", name="d")
            nc.gpsimd.tensor_sub(out=c_, in0=cg1, in1=cg3)         # Y1r
            nc.vector.tensor_add(out=a_, in0=y2[:, b1], in1=cy0)   # Y0+Y2
            nc.vector.tensor_sub(out=b_, in0=cy0, in1=y2[:, b1])   # Y0-Y2
            nc.vector.tensor_add(out=d_, in0=cg1, in1=cg2)         # Y1i
            otv = ot7.rearrange("p (j b) -> p j b", j=4)
            obv = ob_d[YCH - 1].rearrange("p (j b) -> p j b", j=4)
            nc.vector.tensor_sub(out=otv[:, 0, b1], in0=b_, in1=d_)   # o1
            nc.vector.tensor_add(out=otv[:, 1, b1], in0=b_, in1=d_)   # o3
            nc.scalar.dma_start(out=obv[:, 0:2, b1], in_=otv[:, 0:2, b1])
            nc.vector.tensor_add(out=otv[:, 2, b1], in0=a_, in1=c_)   # o0
            nc.vector.tensor_sub(out=otv[:, 3, b1], in0=a_, in1=c_)   # o2
            nc.sync.dma_start(out=obv[:, 2:4, b1], in_=otv[:, 2:4, b1])
    nc.compile()
    return nc


def _prep_eigens(eigens):
    """eigens (gy, gx, 4) -> fp16 pack [YCHP, 128, XC, 5*256]:
    (ychp, x%128, x//128, (k, y%256)) with irfft scales folded in."""
    e = np.ascontiguousarray(np.asarray(eigens).transpose(1, 0, 2)).astype(np.float32)
    e0 = ((e[..., 0] + e[..., 2]) + (e[..., 1] + e[..., 3])) * 0.25
    e2 = ((e[..., 0] + e[..., 2]) - (e[..., 1] + e[..., 3])) * 0.25
    e1r = (e[..., 0] - e[..., 2]) * 0.5
    e1i = (e[..., 3] - e[..., 1]) * 0.5
    # k order: E0, E1r, Ed=E1i-E1r, E2, Es=E1r+E1i
    earr = np.stack([e0, e1r, e1i - e1r, e2, e1r + e1i])        # (5, GX, GY)
    pack = (earr.reshape(5, XC, 128, YCHP, 256)
            .transpose(3, 2, 1, 0, 4)                           # ychp,p,xc,k,y
            .reshape(YCHP, 128, XC, 5 * 256))
    return np.ascontiguousarray(pack).astype(np.float16)


def _in_maps(x, eigens):
    xT = np.asarray(x, dtype=np.float32).T.astype(np.float16)   # [IN, B]
    ep = _prep_eigens(eigens)
    return [
        {"xp": np.ascontiguousarray(xT[:, c * BS:(c + 1) * BS]).reshape(
            XC, 128, 4 * BS),
         "ep": ep}
        for c in range(NCORES)
    ]


def _post(obs):
    """Per-core ob [YCH, 128, 4*BS] fp16 -> full (B, OUT) fp32."""
    out = np.empty((B, OUT), dtype=np.float32)
    for c, ob in enumerate(obs):
        o = np.asarray(ob).reshape(YCH, 128, 4, BS).astype(np.float32)
        o = o[:, :, (2, 0, 3, 1), :]   # undo the (1,3,0,2) j packing
        out[c * BS:(c + 1) * BS] = o.transpose(3, 0, 1, 2).reshape(BS, OUT)
    return out


def kernel(x, eigens):
    from concourse.bass_utils import run_bass_kernel_spmd

    if "nc" not in _cache:
        _cache["nc"] = _build_nc()
    res = run_bass_kernel_spmd(_cache["nc"], _in_maps(x, eigens),
                               core_ids=list(range(NCORES)))
    return _post([r["ob"] for r in res.results])


# revision 20
# speedup vs baseline: 1.0391x; 1.0391x over previous
"""Block-circulant linear (MINI_BLOCK=4) via length-4 rFFT factorization on 8 trn2 cores.

Math: out = x @ W^T where W[4y+n, 4x+j] = eigens[y, x, (n-j) mod 4].
In the length-4 DFT domain the circulant contraction factors into 5 real
matmul chains over the block-index axis gx=1024 (Gauss 3-mult for the complex
bin; FLOP-optimal per Winograd for length-4 cyclic convolution):
  X0 = x0+x1+x2+x3, X1 = (x0-x2) + i(x3-x1), X2 = x0-x1+x2-x3  (per block of 4)
  Y0 = E0^T X0, Y2 = E2^T X2, g1 = E1r^T (X1r+X1i), g2 = Ed^T X1r, g3 = Es^T X1i
  Y1r = g1-g3, Y1i = g1+g2
  o0 = Y0+Y1r+Y2, o1 = Y0-Y1i-Y2, o2 = Y0-Y1r+Y2, o3 = Y0+Y1i-Y2  (scales in E)

Sharding: data-parallel over batch, 512 rows per core; E replicated.

Orientation: E is the matmul *stationary* operand ([128x, 128y] chunks), the
DFT'd x is the *moving* operand ([128x, 512b]); each output group is 128
y-blocks x full batch shard, so a group needs only x + 1/8 of E to retire.
Schedule: x split over three HWDGE rings (sync/scalar/vector) so it outruns
the E stream (gpsimd ring, fine-grained first chunk); 16 dependency-free
warmup matmuls ramp the PE p-state; groups 0+1 interleave 8 chains per
xc-step (exactly 8 PSUM banks) to keep the PE fed during the stream chase;
all PSUM drains are fp16 ACT copies (g2/g3 first so the single-buffered
banks recycle just-in-time); DVE runs fp16 2x-mode butterflies/combines;
the last group runs as two half-batch chain sets so the final epilogue is
half-length and overlaps the other half's matmuls. Output is stored
[y, j, b]-packed fp16 and the host transposes/casts back.
"""
import numpy as np

B, IN, OUT, BLK = 4096, 4096, 4096, 4
GX, GY = IN // BLK, OUT // BLK        # 1024, 1024
NCORES = 8
BS = B // NCORES                      # 512 batch rows per core
XC = GX // 128                        # 8 x-chunks (contraction)
YCH = GY // 128                       # 8 y-groups (128 y-blocks each)
YCHP = YCH // 2                       # 4 E-pack chunks (256 y each)

_cache = {}


def _build_nc():
    from concourse import bacc
    import concourse.mybir as mybir
    from concourse.tile import TileContext

    f32 = mybir.dt.float32
    f16 = mybir.dt.float16

    nc = bacc.Bacc("TRN2", target_bir_lowering=False, debug=False,
                   enable_asserts=False, num_devices=NCORES)
    # x shard host-packed: [XC, 128, 4*BS] = (xc, p, (j b)); row 4*(128*xc+p)+j
    # of x^T lands at [xc, p, j*BS:]. 4KB contiguous per partition per chunk.
    xp_d = nc.dram_tensor("xp", [XC, 128, 4 * BS], f16, kind="ExternalInput")
    # E host-packed in consumption order: [YCHP, 128, XC, 5*256] =
    # (ychp, p=x%128, xc, (k y256)); 2.5KB contiguous runs per partition.
    ep_d = nc.dram_tensor("ep", [YCHP, 128, XC, 5 * 256], f16, kind="ExternalInput")
    # out packed [ych, p=y%128, (j b)] fp16; host transposes to [b, 4y+j].
    ob_d = nc.dram_tensor("ob", [YCH, 128, 4 * BS], f16, kind="ExternalOutput")

    with TileContext(nc) as tc:
        with (
            tc.tile_pool(name="xin", bufs=1) as xip,
            tc.tile_pool(name="xt", bufs=1) as xtp,
            tc.tile_pool(name="esl", bufs=1) as esp,
            tc.tile_pool(name="bfly", bufs=2) as bfp,
            tc.tile_pool(name="drain", bufs=2) as drp,
            tc.tile_pool(name="outp", bufs=2) as op_,
            tc.tile_pool(name="wrm", bufs=1) as wmp,
            tc.tile_pool(name="mpsum", bufs=1, space="PSUM") as mps,
        ):
            xin = xip.tile([128, XC, 4 * BS], f16, tag="xin")
            xt = [xtp.tile([128, XC, BS], f16, tag=f"xt{k}", name=f"xt{k}")
                  for k in range(5)]  # X0, X1s, X1r, X2, X1i
            eslab = esp.tile([128, YCHP, XC, 5 * 256], f16, tag="eslab")

            # --- PE p-state warmup FIRST: dependency-free matmuls on a zeroed
            # tile, alternating the two single-buffered PSUM tags (banks are
            # WAW-recycled by group 0 with no drain needed). The PE ramps
            # 0.65->1.2->2.4GHz over ~3us of busy, so this hides the ramp
            # under the preamble/stream wait.
            wt = wmp.tile([128, 128 + BS], f16, tag="warm")
            nc.gpsimd.memset(wt, 0.0)
            for i in range(18):
                wp = mps.tile([128, BS], f32, tag=("g2" if i % 2 else "g3"),
                              name="wp")
                nc.tensor.matmul(wp, wt[:, :128], wt[:, 128:], start=True, stop=True)

            # --- DMA issues: x and the first E chunk interleaved per-xc in
            # exact phase-1 consumption order, split over the two HWDGE rings
            # (sync even xc, scalar odd xc); the later E chunks follow.
            for xc in range(XC):
                ring = nc.sync if xc % 2 == 0 else nc.scalar
                ring.dma_start(out=xin[:, xc], in_=xp_d[xc])
                ring.dma_start(out=eslab[:, 0, xc], in_=ep_d[0, :, xc])
            for ring, ychp, xlo, xhi in ((nc.sync, 1, 0, 2), (nc.scalar, 1, 2, 4),
                                         (nc.sync, 1, 4, 6), (nc.scalar, 1, 6, 8),
                                         (nc.sync, 2, 0, 8), (nc.scalar, 3, 0, 8)):
                ring.dma_start(out=eslab[:, ychp, xlo:xhi],
                               in_=ep_d[ychp, :, xlo:xhi])

            # --- Forward DFT butterflies, fp16 DVE, chasing the x stream.
            for xc in range(XC):
                xj = [xin[:, xc, j * BS:(j + 1) * BS] for j in range(4)]
                s02 = bfp.tile([128, BS], f16, tag="s02")
                s13 = bfp.tile([128, BS], f16, tag="s13")
                nc.vector.tensor_add(out=s02, in0=xj[0], in1=xj[2])
                nc.vector.tensor_add(out=s13, in0=xj[1], in1=xj[3])
                nc.vector.tensor_sub(out=xt[2][:, xc], in0=xj[0], in1=xj[2])
                nc.vector.tensor_sub(out=xt[4][:, xc], in0=xj[3], in1=xj[1])
                nc.vector.tensor_add(out=xt[0][:, xc], in0=s02, in1=s13)
                nc.vector.tensor_sub(out=xt[3][:, xc], in0=s02, in1=s13)
                nc.vector.tensor_add(out=xt[1][:, xc], in0=xt[2][:, xc], in1=xt[4][:, xc])

            # --- Main loop helpers. Chain order i: y0, g1, y2, g2, g3;
            # E-matrix pack order k: E0, E1r, Ed, E2, Es; chain i uses E-matrix
            # kmap[i] against xt[kmap'[i]] (same permutation both sides).
            kmap = (0, 1, 3, 2, 4)

            def alloc_g23():
                return (mps.tile([128, BS], f32, tag="g2", name="g2"),
                        mps.tile([128, BS], f32, tag="g3", name="g3"))

            def alloc_chains(dbl_only=False):
                y0 = mps.tile([128, BS], f32, tag="y0", bufs=2)
                g1 = mps.tile([128, BS], f32, tag="g1", bufs=2)
                y2 = mps.tile([128, BS], f32, tag="y2", bufs=2)
                if dbl_only:
                    return (y0, g1, y2)
                return (y0, g1, y2) + alloc_g23()

            def mm(chains, ych, i, xc, bsl, start, stop):
                k = kmap[i]
                lo = k * 256 + (ych % 2) * 128
                nc.tensor.matmul(chains[i][:, bsl],
                                 eslab[:, ych // 2, xc, lo:lo + 128],
                                 xt[k][:, xc, bsl], start=start, stop=stop)

            BSL = slice(0, BS)

            def emit_chains(chains, ych, bsl=BSL, idxs=None):
                idxs = range(len(chains)) if idxs is None else idxs
                # first two xc steps run double-buffered chains first so the
                # PE never waits on the prior group's g2/g3 drains
                for xc in (0, 1):
                    for i in idxs:
                        if i < 3:
                            mm(chains, ych, i, xc, bsl, xc == 0, False)
                for xc in (0, 1):
                    for i in idxs:
                        if i >= 3:
                            mm(chains, ych, i, xc, bsl, xc == 0, False)
                for xc in range(2, XC):
                    for i in idxs:
                        mm(chains, ych, i, xc, bsl, False, xc == XC - 1)

            def emit_epi(chains, ych, bsl=BSL, ot=None):
                y0, g1, y2, g2, g3 = chains
                n = bsl.stop - bsl.start
                cv = {}
                # ACT drains: single-buffered banks first, then DVE deps
                for nm, src in (("cg2", g2), ("cg3", g3), ("cy2", y2),
                                ("cy0", y0), ("cg1", g1)):
                    cv[nm] = drp.tile([128, n], f16, tag=nm, name=nm)
                    nc.scalar.copy(out=cv[nm], in_=src[:, bsl])
                a_ = drp.tile([128, n], f16, tag="a")
                b_ = drp.tile([128, n], f16, tag="b")
                c_ = drp.tile([128, n], f16, tag="c")
                d_ = drp.tile([128, n], f16, tag="d")
                nc.vector.tensor_sub(out=c_, in0=cv["cg1"], in1=cv["cg3"])  # Y1r
                nc.vector.tensor_add(out=d_, in0=cv["cg1"], in1=cv["cg2"])  # Y1i
                nc.vector.tensor_add(out=a_, in0=cv["cy0"], in1=cv["cy2"])  # Y0+Y2
                nc.vector.tensor_sub(out=b_, in0=cv["cy0"], in1=cv["cy2"])  # Y0-Y2
                if ot is None:
                    ot = op_.tile([128, 4 * BS], f16, tag="ot")
                otv = ot.rearrange("p (j b) -> p j b", j=4)
                # j planes packed in order (1, 3, 0, 2) so the tail's two
                # store halves are contiguous; _post undoes the permutation
                nc.vector.tensor_add(out=otv[:, 2, bsl], in0=a_, in1=c_)  # o0
                nc.vector.tensor_sub(out=otv[:, 3, bsl], in0=a_, in1=c_)  # o2
                nc.vector.tensor_sub(out=otv[:, 0, bsl], in0=b_, in1=d_)  # o1
                nc.vector.tensor_add(out=otv[:, 1, bsl], in0=b_, in1=d_)  # o3
                if n == BS:
                    nc.sync.dma_start(out=ob_d[ych], in_=ot)
                else:
                    nc.sync.dma_start(
                        out=ob_d[ych].rearrange("p (j b) -> p j b", j=4)[:, :, bsl],
                        in_=otv[:, :, bsl])
                return ot

            # --- Groups 0+1 interleaved: 8 chains per xc-step (8 PSUM banks)
            # so the PE keeps pace with the x/E stream; then ych1's g2/g3.
            # inner order follows butterfly completion (X1r, X1i land first,
            # X1s last) so the first matmul issues as early as possible
            ch0 = alloc_chains()
            ch1d = alloc_chains(dbl_only=True)
            for xc in range(XC):
                st, sp = xc == 0, xc == XC - 1
                for i in (3, 4, 0, 2, 1):
                    mm(ch0, 0, i, xc, BSL, st, sp)
                for i in (0, 2, 1):
                    mm(ch1d, 1, i, xc, BSL, st, sp)
            emit_epi(ch0, 0)
            ch1 = ch1d + alloc_g23()
            emit_chains(ch1, 1, idxs=(3, 4))
            emit_epi(ch1, 1)

            # --- Groups 2..6 plain; group 7 as two half-batch chain sets so
            # the tail epilogue is half-length and overlaps h1's matmuls.
            for ych in range(2, YCH - 1):
                ch = alloc_chains()
                emit_chains(ch, ych)
                emit_epi(ch, ych)
            b0, b1 = slice(0, BS // 2), slice(BS // 2, BS)
            ch = alloc_chains()
            emit_chains(ch, YCH - 1, bsl=b0)
            ot7 = emit_epi(ch, YCH - 1, bsl=b0)
            # h1 tail: chain-major so y0/g1 stop early and drain under the
            # y2/g2/g3 phase; a/b read y2 straight from PSUM. Shortens the
            # post-last-matmul critical path to ~2us + store.
            y0, g1, y2, g2, g3 = alloc_chains()
            ch = (y0, g1, y2, g2, g3)
            for xc in range(XC):
                mm(ch, YCH - 1, 0, xc, b1, xc == 0, xc == XC - 1)
                mm(ch, YCH - 1, 1, xc, b1, xc == 0, xc == XC - 1)
            n2 = BS // 2
            cg1 = drp.tile([128, n2], f16, tag="cg1", name="cg1")
            cy0 = drp.tile([128, n2], f16, tag="cy0", name="cy0")
            nc.scalar.copy(out=cg1, in_=g1[:, b1])
            nc.scalar.copy(out=cy0, in_=y0[:, b1])
            # y2 last in the cycle so g3/g2 stop (and drain) before the final
            # matmul; DVE handles the b/d leg while gpsimd does the c leg.
            for xc in range(XC):
                for i in (4, 3, 2):
                    mm(ch, YCH - 1, i, xc, b1, xc == 0, xc == XC - 1)
            cg3 = drp.tile([128, n2], f16, tag="cg3", name="cg3")
            cg2 = drp.tile([128, n2], f16, tag="cg2", name="cg2")
            nc.scalar.copy(out=cg3, in_=g3[:, b1])
            nc.scalar.copy(out=cg2, in_=g2[:, b1])
            a_ = drp.tile([128, n2], f16, tag="a", name="a")
            b_ = drp.tile([128, n2], f16, tag="b", name="b")
            c_ = drp.tile([128, n2], f16, tag="c", name="c")
            d_ = drp.tile([128, n2], f16, tag="d", name="d")
            nc.gpsimd.tensor_sub(out=c_, in0=cg1, in1=cg3)         # Y1r
            nc.vector.tensor_add(out=a_, in0=y2[:, b1], in1=cy0)   # Y0+Y2
            nc.vector.tensor_sub(out=b_, in0=cy0, in1=y2[:, b1])   # Y0-Y2
            nc.vector.tensor_add(out=d_, in0=cg1, in1=cg2)         # Y1i
            otv = ot7.rearrange("p (j b) -> p j b", j=4)
            obv = ob_d[YCH - 1].rearrange("p (j b) -> p j b", j=4)
            nc.vector.tensor_sub(out=otv[:, 0, b1], in0=b_, in1=d_)   # o1
            nc.vector.tensor_add(out=otv[:, 1, b1], in0=b_, in1=d_)   # o3
            nc.scalar.dma_start(out=obv[:, 0:2, b1], in_=otv[:, 0:2, b1])
            nc.vector.tensor_add(out=otv[:, 2, b1], in0=a_, in1=c_)   # o0
            nc.vector.tensor_sub(out=otv[:, 3, b1], in0=a_, in1=c_)   # o2
            nc.sync.dma_start(out=obv[:, 2:4, b1], in_=otv[:, 2:4, b1])
    nc.compile()
    return nc


def _prep_eigens(eigens):
    """eigens (gy, gx, 4) -> fp16 pack [YCHP, 128, XC, 5*256]:
    (ychp, x%128, x//128, (k, y%256)) with irfft scales folded in."""
    e = np.ascontiguousarray(np.asarray(eigens).transpose(1, 0, 2)).astype(np.float32)
    e0 = ((e[..., 0] + e[..., 2]) + (e[..., 1] + e[..., 3])) * 0.25
    e2 = ((e[..., 0] + e[..., 2]) - (e[..., 1] + e[..., 3])) * 0.25
    e1r = (e[..., 0] - e[..., 2]) * 0.5
    e1i = (e[..., 3] - e[..., 1]) * 0.5
    # k order: E0, E1r, Ed=E1i-E1r, E2, Es=E1r+E1i
    earr = np.stack([e0, e1r, e1i - e1r, e2, e1r + e1i])        # (5, GX, GY)
    pack = (earr.reshape(5, XC, 128, YCHP, 256)
            .transpose(3, 2, 1, 0, 4)                           # ychp,p,xc,k,y
            .reshape(YCHP, 128, XC, 5 * 256))
    return np.ascontiguousarray(pack).astype(np.float16)


def _in_maps(x, eigens):
    xT = np.asarray(x, dtype=np.float32).T.astype(np.float16)   # [IN, B]
    ep = _prep_eigens(eigens)
    return [
        {"xp": np.ascontiguousarray(xT[:, c * BS:(c + 1) * BS]).reshape(
            XC, 128, 4 * BS),
         "ep": ep}
        for c in range(NCORES)
    ]


def _post(obs):
    """Per-core ob [YCH, 128, 4*BS] fp16 -> full (B, OUT) fp32."""
    out = np.empty((B, OUT), dtype=np.float32)
    for c, ob in enumerate(obs):
        o = np.asarray(ob).reshape(YCH, 128, 4, BS).astype(np.float32)
        o = o[:, :, (2, 0, 3, 1), :]   # undo the (1,3,0,2) j packing
        out[c * BS:(c + 1) * BS] = o.transpose(3, 0, 1, 2).reshape(BS, OUT)
    return out


def kernel(x, eigens):
    from concourse.bass_utils import run_bass_kernel_spmd

    if "nc" not in _cache:
        _cache["nc"] = _build_nc()
    res = run_bass_kernel_spmd(_cache["nc"], _in_maps(x, eigens),
                               core_ids=list(range(NCORES)))
    return _post([r["ob"] for r in res.results])
